# revision 1
# baseline (speedup 1.0000x reference)
"""Causal MHSA with RoPE on 8 TRN2 NeuronCores (head-parallel, 2 heads/core).

Self-contained: hardcodes shapes (b=1, s=4096, d_model=1024, 16 heads, hs=64).

Per-core dataflow (all matmuls float32r = 4x-rate fp32, ~1.5e-4 rounding):
  1. QKV projection into transposed layout qT/kT/vT [e, s] (e on partitions),
     streaming RoPE on q/k (pair-swap stream_shuffle formulation), PE-transpose
     of V into [s, d] tiles with a fused ones-column per head for the softmax
     denominator.
  2. Attention with scores computed transposed: S^T[j, i] = k_j . q_i so the
     softmax needs no transposes. Causal mask added on PE via an identity
     matmul of a precomputed -1e9 mask into PSUM before the score matmul.
     exp() batched over two j-chunks [128, 1024] to amortize the ACT access
     bubble; no max-subtraction (scores are bounded ~ +-4 here, exp is safe
     in fp32). The AV matmul's 65th lhsT column of ones accumulates the
     denominator for free; normalization happens after AV via reciprocal +
     gpsimd partition-broadcast.
  3. Per-512-query-chunk output projection with this core's 128 W_o columns;
     the 8 partial [1024, s] outputs are summed on the host.

  QKV(n) -> RoPE(n) -> attention(n) -> projection(n) run in ONE interleaved
  loop with a single coexisting PSUM pool set (qkv 1 + vtr 1 + scores 2x2 +
  out-accum 1 + proj 1 = 8 banks), so the tensor engine fills ACT-gated
  attention stalls with QKV work for later chunks and attention starts
  ~24us in instead of after the whole DMA-bound projection phase.
"""

import numpy as np

DM = 1024
NH = 16
HS = 64
NCORES = 8
THETA = 10000.0
S = 4096
NB = 512
JB = 128
GRP = 2
MASK = True


def _build(s_len):
    import concourse.bass as bass
    import concourse.mybir as mybir
    import concourse.tile as tile
    from concourse import bacc
    from contextlib import ExitStack

    f32 = mybir.dt.float32
    f32r = mybir.dt.float32r
    Exp = mybir.ActivationFunctionType.Exp

    n_nb = s_len // NB
    n_jb = s_len // JB
    jb_per_nb = NB // JB

    nc = bacc.Bacc("TRN2", target_bir_lowering=False, debug=False,
                   num_devices=NCORES)

    xT = nc.dram_tensor("xT", [DM, s_len], f32r, kind="ExternalInput").ap()
    wqkvT = nc.dram_tensor("wqkvT", [DM, 3 * 128], f32r,
                           kind="ExternalInput").ap()
    woT = nc.dram_tensor("woT", [128, DM], f32r, kind="ExternalInput").ap()
    cosf = nc.dram_tensor("cosf", [128, s_len], f32, kind="ExternalInput").ap()
    sinf = nc.dram_tensor("sinf", [128, s_len], f32, kind="ExternalInput").ap()
    outT = nc.dram_tensor("outT", [DM, s_len], f32, kind="ExternalOutput").ap()

    shuffle_mask = [r ^ 1 for r in range(32)]

    with tile.TileContext(nc) as tc, ExitStack() as ctx:
        const = ctx.enter_context(tc.tile_pool(name="const", bufs=1))
        slabs = ctx.enter_context(tc.tile_pool(name="slabs", bufs=1))

        zeros_f32 = const.tile([128, 128], f32, tag="zeros_f32")
        nc.gpsimd.memset(zeros_f32[:], 0.0)
        ones_f32 = const.tile([128, 1], f32, tag="ones_f32")
        nc.gpsimd.memset(ones_f32[:], 1.0)
        ident = const.tile([128, 128], f32r, tag="ident")
        nc.scalar.copy(ident[:], zeros_f32[:])
        nc.gpsimd.affine_select(
            out=ident[:], in_=ident[:],
            compare_op=mybir.AluOpType.not_equal, fill=1.0,
            base=0, pattern=[[-1, 128]], channel_multiplier=1)

        masks = const.tile([128, 4, NB], f32r, tag="masks")
        zl = const.tile([128, NB], f32, tag="zl")
        nc.gpsimd.memset(zl[:], 0.0)
        for dm in range(4):
            nc.scalar.copy(masks[:, dm, :], zl[:])
            nc.gpsimd.affine_select(
                out=masks[:, dm, :], in_=masks[:, dm, :],
                compare_op=mybir.AluOpType.is_ge, fill=-1e9,
                base=-128 * dm, pattern=[[1, NB]], channel_multiplier=-1)

        w_sb = const.tile([128, 8, 384], f32r, tag="w_sb")
        for k in range(8):
            nc.sync.dma_start(w_sb[:, k, :], wqkvT[128 * k:128 * (k + 1), :])
        wo_sb = const.tile([128, DM], f32r, tag="wo_sb")

        qT = slabs.tile([128, s_len], f32r, tag="qT")
        kT = slabs.tile([128, s_len], f32r, tag="kT")
        v1 = slabs.tile([128, n_jb, 130], f32r, tag="v1")
        oT = slabs.tile([128, s_len], f32r, tag="oT")

        with tc.tile_pool(name="xp", bufs=12) as xp, \
             tc.tile_pool(name="qkv_ps", bufs=1, space="PSUM") as qkv_ps, \
             tc.tile_pool(name="tr_ps", bufs=1, space="PSUM") as tr_ps, \
             tc.tile_pool(name="s_ps", bufs=2, space="PSUM") as s_ps, \
             tc.tile_pool(name="o_ps", bufs=1, space="PSUM") as o_ps, \
             tc.tile_pool(name="pr_ps", bufs=1, space="PSUM") as pr_ps, \
             tc.tile_pool(name="rtmp", bufs=3) as rtmp, \
             tc.tile_pool(name="csp", bufs=3) as csp, \
             tc.tile_pool(name="pp", bufs=6) as pp, \
             tc.tile_pool(name="ntmp", bufs=4) as ntmp, \
             tc.tile_pool(name="ostg", bufs=8) as ostg, \
             tc.tile_pool(name="vtmp", bufs=2) as vtmp:
            nc.sync.dma_start(wo_sb[:], woT[:, :])
            for n in range(n_nb):
                xts = []
                for k in range(8):
                    xt = xp.tile([128, NB], f32r, tag="xt")
                    nc.sync.dma_start(
                        xt[:], xT[128 * k:128 * (k + 1), NB * n:NB * (n + 1)])
                    xts.append(xt)
                cos_t = csp.tile([128, NB], f32, tag="cos_t")
                nc.sync.dma_start(cos_t[:], cosf[:, NB * n:NB * (n + 1)])
                sin_t = csp.tile([128, NB], f32, tag="sin_t")
                nc.sync.dma_start(sin_t[:], sinf[:, NB * n:NB * (n + 1)])
                vt_n = vtmp.tile([128, NB], f32r, tag="vt")
                for m in range(3):
                    ps = qkv_ps.tile([128, NB], f32)
                    for k in range(8):
                        nc.tensor.matmul(ps[:], w_sb[:, k, 128 * m:128 * (m + 1)],
                                         xts[k][:], start=(k == 0), stop=(k == 7))
                    if m == 2:
                        nc.scalar.copy(vt_n[:], ps[:])
                    else:
                        dst = qT if m == 0 else kT
                        cs = cos_t[:]
                        sn = sin_t[:]
                        shuf = rtmp.tile([128, NB], f32, tag="shuf")
                        nc.vector.stream_shuffle(shuf[:], ps[:], shuffle_mask)
                        t0 = rtmp.tile([128, NB], f32, tag="t0")
                        nc.vector.tensor_mul(t0[:], ps[:], cs)
                        t1 = rtmp.tile([128, NB], f32, tag="t1")
                        nc.vector.tensor_mul(t1[:], shuf[:], sn)
                        nc.vector.tensor_add(dst[:, NB * n:NB * (n + 1)],
                                             t0[:], t1[:])
                for jj in range(jb_per_nb):
                    j = jb_per_nb * n + jj
                    tp = tr_ps.tile([128, 128], f32r)
                    for h in range(2):
                        nc.tensor.transpose(
                            tp[:, 64 * h:64 * (h + 1)],
                            vt_n[64 * h:64 * (h + 1), 128 * jj:128 * (jj + 1)],
                            ident[64 * h:64 * (h + 1), 64 * h:64 * (h + 1)])
                        nc.scalar.copy(v1[:, j, 65 * h:65 * h + 64],
                                       tp[:, 64 * h:64 * (h + 1)])
                        nc.scalar.copy(v1[:, j, 65 * h + 64:65 * h + 65],
                                       ones_f32[:])

                # ---- attention + projection for chunk n ----
                n_grp = (n + 1) * jb_per_nb // GRP
                for h in range(2):
                    op = o_ps.tile([65, NB], f32)
                    for g in range(n_grp):
                        sp = s_ps.tile([128, GRP, NB], f32)
                        dm0 = GRP * g - jb_per_nb * n
                        for ms in range(GRP):
                            m = GRP * g + ms
                            diag = MASK and 0 <= dm0 + ms
                            if diag:
                                nc.tensor.matmul(
                                    sp[:, ms, :], ident[:],
                                    masks[:, dm0 + ms, :],
                                    start=True, stop=False)
                            nc.tensor.matmul(
                                sp[:, ms, :],
                                kT[64 * h:64 * (h + 1), 128 * m:128 * (m + 1)],
                                qT[64 * h:64 * (h + 1), NB * n:NB * (n + 1)],
                                start=not diag, stop=True)
                        p = pp.tile([128, GRP, NB], f32r, tag="p")
                        nc.scalar.activation(p[:], sp[:], Exp, scale=0.125)
                        for ms in range(GRP):
                            m = GRP * g + ms
                            nc.tensor.matmul(
                                op[:], v1[:, m, 65 * h:65 * h + 65],
                                p[:, ms, :], start=(m == 0),
                                stop=(m == GRP * n_grp - 1))
                    recip = ntmp.tile([1, NB], f32, tag="recip")
                    nc.vector.reciprocal(recip[:], op[64:65, :])
                    bc = ntmp.tile([64, NB], f32, tag="bc")
                    nc.gpsimd.partition_broadcast(bc[:], recip[:])
                    nc.vector.tensor_mul(
                        oT[64 * h:64 * (h + 1), NB * n:NB * (n + 1)],
                        op[0:64, :], bc[:])
                for me in range(8):
                    prp = pr_ps.tile([128, NB], f32)
                    nc.tensor.matmul(prp[:], wo_sb[:, 128 * me:128 * (me + 1)],
                                     oT[:, NB * n:NB * (n + 1)],
                                     start=True, stop=True)
                    ot = ostg.tile([128, NB], f32, tag="ot")
                    nc.vector.tensor_copy(ot[:], prp[:])
                    nc.sync.dma_start(
                        outT[128 * me:128 * (me + 1), NB * n:NB * (n + 1)],
                        ot[:])

    nc.compile()
    return nc


_CACHE = {}


def _get_nc(s_len):
    if s_len not in _CACHE:
        _CACHE[s_len] = _build(s_len)
    return _CACHE[s_len]


def _host_inputs(x, token_positions, W_qkv, W_o, s_len):
    xT = np.ascontiguousarray(x.reshape(s_len, DM).T).astype(np.float32)
    pos = token_positions.astype(np.float32)
    kk = np.arange(HS // 2, dtype=np.float32)
    inv_freq = 1.0 / (THETA ** (2.0 * kk / HS))
    ang = pos[:, None] * inv_freq[None, :]
    cos = np.repeat(np.cos(ang), 2, axis=1).T        # [64, s]
    sin = np.repeat(np.sin(ang), 2, axis=1).T        # [64, s]
    sgn = np.where(np.arange(HS) % 2 == 0, -1.0, 1.0).astype(np.float32)
    sinm = sin * sgn[:, None]
    cosf = np.ascontiguousarray(np.concatenate([cos, cos], 0)).astype(np.float32)
    sinf = np.ascontiguousarray(np.concatenate([sinm, sinm], 0)).astype(np.float32)

    in_maps = []
    for c in range(NCORES):
        r0 = 128 * c
        wc = np.concatenate([W_qkv[r0:r0 + 128],
                             W_qkv[DM + r0:DM + r0 + 128],
                             W_qkv[2 * DM + r0:2 * DM + r0 + 128]], 0)
        wqkvT = np.ascontiguousarray(wc.T).astype(np.float32)
        woT = np.ascontiguousarray(W_o[:, r0:r0 + 128].T).astype(np.float32)
        in_maps.append(dict(xT=xT, wqkvT=wqkvT, woT=woT, cosf=cosf, sinf=sinf))
    return in_maps


def run_on_device(x, token_positions, W_qkv, W_o, s_len=S, trace=False):
    from concourse.bass_utils import run_bass_kernel_spmd
    nc = _get_nc(s_len)
    in_maps = _host_inputs(np.asarray(x), np.asarray(token_positions),
                           np.asarray(W_qkv), np.asarray(W_o), s_len)
    # The axon-tunneled devices intermittently fault with
    # NRT_EXEC_UNIT_UNRECOVERABLE (observed even on trivial known-good
    # kernels); a retry on a fresh attempt reliably recovers.
    last_err = None
    for _attempt in range(3):
        try:
            res = run_bass_kernel_spmd(nc, in_maps,
                                       core_ids=list(range(NCORES)),
                                       trace=trace)
            break
        except Exception as e:  # jax.errors.JaxRuntimeError
            last_err = e
    else:
        raise last_err
    acc = np.zeros((DM, s_len), dtype=np.float64)
    for r in res.results:
        acc += r["outT"].astype(np.float64)
    out = acc.T.astype(np.float32).reshape(1, s_len, DM)
    return out, res


def kernel(x, token_positions, W_qkv, W_o):
    x = np.asarray(x)
    b, s_len, _ = x.shape
    assert b == 1
    out, _ = run_on_device(x, token_positions, W_qkv, W_o, s_len=s_len)
    return out



# revision 2
# speedup vs baseline: 6932.6035x; 6932.6035x over previous
"""Causal MHSA with RoPE on 8 TRN2 NeuronCores (head-parallel, fp16 compute).

Self-contained: hardcodes shapes (b=1, s=4096, d_model=1024, 16 heads, hs=64).

Per-core dataflow (2 heads/core; all PE operands fp16, PSUM accumulate f32):
  1. QKV projection into transposed layout qT/kT/vT [e, s] from the full
     (device-replicated) xT, streaming RoPE on q/k (pair-swap stream_shuffle),
     PE-transpose of V into [s, d] tiles with a fused ones-column per head.
  2. Attention with scores computed transposed: S^T[j, i] = k_j . q_i, causal
     mask added on PE via an identity-matmul of a precomputed -3e4 mask into
     PSUM before the score matmul, exp() batched over two j-chunks on ACT,
     the AV matmul's 65th lhsT column of ones accumulating the softmax
     denominator for free; normalization on DVE from an SBUF copy.
  3. Partial output projection over this core's 128 o-dims, emitted in [s, e]
     orientation as fp16, so the cross-core reduction is a plain sum of the 8
     partial outputs (done on device via an XLA psum; host fallback: np sum).

Distribution glue (outside the NEFF, cached across calls):
  - x is uploaded once per call as 8 row-shards of xT (1 MB/core) and
    replicated on-device by an XLA all_gather.
  - weights / RoPE tables are uploaded once and cached on device, keyed by
    value equality with the previous call's arrays.
"""

import numpy as np

DM = 1024
NH = 16
HS = 64
NCORES = 8
THETA = 10000.0
S = 4096
GRP = 2
NEG = -30000.0


def _build(s_len):
    import concourse.mybir as mybir
    import concourse.tile as tile
    from concourse import bacc
    from contextlib import ExitStack

    f32 = mybir.dt.float32
    f16 = mybir.dt.float16
    Exp = mybir.ActivationFunctionType.Exp

    NB = min(512, s_len)
    n_nb = s_len // NB
    jb_per_nb = NB // 128
    n_jb = s_len // 128
    EC = min(512, DM)

    nc = bacc.Bacc("TRN2", target_bir_lowering=False, debug=False,
                   num_devices=NCORES)

    xT = nc.dram_tensor("xT", [DM, s_len], f16, kind="ExternalInput").ap()
    wqkvT = nc.dram_tensor("wqkvT", [DM, 3 * 128], f16,
                           kind="ExternalInput").ap()
    woT = nc.dram_tensor("woT", [128, DM], f16, kind="ExternalInput").ap()
    cosf = nc.dram_tensor("cosf", [128, s_len], f32, kind="ExternalInput").ap()
    sinf = nc.dram_tensor("sinf", [128, s_len], f32, kind="ExternalInput").ap()
    outP = nc.dram_tensor("outP", [s_len, DM], f16, kind="ExternalOutput").ap()

    shuffle_mask = [r ^ 1 for r in range(32)]

    with tile.TileContext(nc) as tc, ExitStack() as ctx:
        const = ctx.enter_context(tc.tile_pool(name="const", bufs=1))
        slabs = ctx.enter_context(tc.tile_pool(name="slabs", bufs=1))

        zeros16 = const.tile([128, NB], f16, tag="zeros16")
        nc.gpsimd.memset(zeros16[:], 0.0)
        ident = const.tile([128, 128], f16, tag="ident")
        nc.vector.tensor_copy(ident[:], zeros16[:, 0:128])
        nc.gpsimd.affine_select(
            out=ident[:], in_=ident[:],
            compare_op=mybir.AluOpType.not_equal, fill=1.0,
            base=0, pattern=[[-1, 128]], channel_multiplier=1)

        masks = const.tile([128, jb_per_nb, NB], f16, tag="masks")
        for dm in range(jb_per_nb):
            nc.vector.tensor_copy(masks[:, dm, :], zeros16[:])
            nc.gpsimd.affine_select(
                out=masks[:, dm, :], in_=masks[:, dm, :],
                compare_op=mybir.AluOpType.is_ge, fill=NEG,
                base=-128 * dm, pattern=[[1, NB]], channel_multiplier=-1)

        w_sb = const.tile([128, 8, 384], f16, tag="w_sb")
        for k in range(8):
            nc.sync.dma_start(w_sb[:, k, :], wqkvT[128 * k:128 * (k + 1), :])
        wo_sb = const.tile([128, DM], f16, tag="wo_sb")
        nc.sync.dma_start(wo_sb[:], woT[:, :])
        # per-chunk table loads (inside the loop) keep the first QKV matmul
        # off the critical path of these 2MB transfers
        cos_sb = const.tile([128, s_len], f32, tag="cos_sb")
        sin_sb = const.tile([128, s_len], f32, tag="sin_sb")

        qT = slabs.tile([128, s_len], f16, tag="qT")
        kT = slabs.tile([128, s_len], f16, tag="kT")
        v1 = slabs.tile([128, n_jb, 130], f16, tag="v1")
        oT = slabs.tile([128, s_len], f16, tag="oT")
        nc.gpsimd.memset(v1[:, :, 64:65], 1.0)
        nc.gpsimd.memset(v1[:, :, 129:130], 1.0)

        with tc.tile_pool(name="xt", bufs=2) as xt_p, \
             tc.tile_pool(name="qkv_ps", bufs=1, space="PSUM") as qkv_ps, \
             tc.tile_pool(name="tr_ps", bufs=1, space="PSUM") as tr_ps, \
             tc.tile_pool(name="s_ps", bufs=2, space="PSUM") as s_ps, \
             tc.tile_pool(name="o_ps", bufs=1, space="PSUM") as o_ps, \
             tc.tile_pool(name="pr_ps", bufs=1, space="PSUM") as pr_ps, \
             tc.tile_pool(name="rtmp", bufs=3) as rtmp, \
             tc.tile_pool(name="pp", bufs=6) as pp_p, \
             tc.tile_pool(name="ntmp", bufs=4) as ntmp, \
             tc.tile_pool(name="ostg", bufs=8) as ostg, \
             tc.tile_pool(name="vtmp", bufs=2) as vtmp:

            for n in range(n_nb):
                # QKV projection for chunk n
                xt = xt_p.tile([128, 8, NB], f16, tag="xt")
                for k in range(8):
                    nc.sync.dma_start(
                        xt[:, k, :],
                        xT[128 * k:128 * (k + 1), NB * n:NB * (n + 1)])
                nc.sync.dma_start(cos_sb[:, NB * n:NB * (n + 1)],
                                  cosf[:, NB * n:NB * (n + 1)])
                nc.sync.dma_start(sin_sb[:, NB * n:NB * (n + 1)],
                                  sinf[:, NB * n:NB * (n + 1)])
                vt = vtmp.tile([128, NB], f16, tag="vt")
                for m in range(3):
                    ps = qkv_ps.tile([128, NB], f32)
                    for k in range(8):
                        nc.tensor.matmul(ps[:], w_sb[:, k, 128 * m:128 * (m + 1)],
                                         xt[:, k, :], start=(k == 0),
                                         stop=(k == 7))
                    if m == 2:
                        nc.vector.tensor_copy(vt[:], ps[:])
                    else:
                        dst = qT if m == 0 else kT
                        cs = cos_sb[:, NB * n:NB * (n + 1)]
                        sn = sin_sb[:, NB * n:NB * (n + 1)]
                        shuf = rtmp.tile([128, NB], f32, tag="shuf")
                        nc.vector.stream_shuffle(shuf[:], ps[:], shuffle_mask)
                        t0 = rtmp.tile([128, NB], f32, tag="t0")
                        nc.vector.tensor_mul(t0[:], ps[:], cs)
                        t1 = rtmp.tile([128, NB], f32, tag="t1")
                        nc.vector.tensor_mul(t1[:], shuf[:], sn)
                        nc.vector.tensor_add(dst[:, NB * n:NB * (n + 1)],
                                             t0[:], t1[:])
                for jj in range(jb_per_nb):
                    j = jb_per_nb * n + jj
                    tp = tr_ps.tile([128, 128], f16)
                    for h in range(2):
                        nc.tensor.transpose(
                            tp[:, 64 * h:64 * (h + 1)],
                            vt[64 * h:64 * (h + 1), 128 * jj:128 * (jj + 1)],
                            ident[64 * h:64 * (h + 1), 64 * h:64 * (h + 1)])
                        nc.vector.tensor_copy(v1[:, j, 65 * h:65 * h + 64],
                                              tp[:, 64 * h:64 * (h + 1)])

                # attention for chunk n, both heads
                n_j = (n + 1) * jb_per_nb
                for h in range(2):
                    op = o_ps.tile([65, NB], f32)
                    for g0 in range(0, n_j, GRP):
                        grp = list(range(g0, min(g0 + GRP, n_j)))
                        sp = s_ps.tile([128, GRP, NB], f32)
                        for ms, m in enumerate(grp):
                            dmo = m - jb_per_nb * n
                            diag = dmo >= 0
                            if diag:
                                nc.tensor.matmul(
                                    sp[:, ms, :], ident[:], masks[:, dmo, :],
                                    start=True, stop=False)
                            nc.tensor.matmul(
                                sp[:, ms, :],
                                kT[64 * h:64 * (h + 1), 128 * m:128 * (m + 1)],
                                qT[64 * h:64 * (h + 1), NB * n:NB * (n + 1)],
                                start=not diag, stop=True)
                        p = pp_p.tile([128, GRP, NB], f16, tag="p")
                        nc.scalar.activation(p[:, 0:len(grp), :],
                                             sp[:, 0:len(grp), :],
                                             Exp, scale=0.125)
                        for ms, m in enumerate(grp):
                            nc.tensor.matmul(
                                op[:], v1[:, m, 65 * h:65 * h + 65],
                                p[:, ms, :], start=(m == 0),
                                stop=(m == n_j - 1))
                    obs = ntmp.tile([65, NB], f32, tag="obs")
                    nc.vector.tensor_copy(obs[:], op[:])
                    recip = ntmp.tile([1, NB], f32, tag="recip")
                    nc.vector.reciprocal(recip[:], obs[64:65, :])
                    bc = ntmp.tile([64, NB], f32, tag="bc")
                    nc.gpsimd.partition_broadcast(bc[:], recip[:])
                    nc.vector.tensor_mul(
                        oT[64 * h:64 * (h + 1), NB * n:NB * (n + 1)],
                        obs[0:64, :], bc[:])

                # partial projection for chunk n: out[s, e] += oT_c.T @ woT_c
                for sc in range(NB // 128):
                    for ec in range(DM // EC):
                        prp = pr_ps.tile([128, EC], f32)
                        nc.tensor.matmul(
                            prp[:],
                            oT[:, NB * n + 128 * sc:NB * n + 128 * (sc + 1)],
                            wo_sb[:, EC * ec:EC * (ec + 1)],
                            start=True, stop=True)
                        ot = ostg.tile([128, EC], f16, tag="ot")
                        nc.vector.tensor_copy(ot[:], prp[:])
                        nc.sync.dma_start(
                            outP[NB * n + 128 * sc:NB * n + 128 * (sc + 1),
                                 EC * ec:EC * (ec + 1)],
                            ot[:])

    nc.compile()
    return nc


def _rope_tables(token_positions, s_len):
    pos = np.asarray(token_positions).astype(np.float64)
    kk = np.arange(HS // 2, dtype=np.float64)
    inv_freq = 1.0 / (THETA ** (2.0 * kk / HS))
    ang = pos[:, None] * inv_freq[None, :]
    cos = np.repeat(np.cos(ang), 2, axis=1).T.astype(np.float32)   # [64, s]
    sin = np.repeat(np.sin(ang), 2, axis=1).T.astype(np.float32)
    sgn = np.where(np.arange(HS) % 2 == 0, -1.0, 1.0).astype(np.float32)
    sinm = sin * sgn[:, None]
    cosf = np.ascontiguousarray(np.concatenate([cos, cos], 0))
    sinf = np.ascontiguousarray(np.concatenate([sinm, sinm], 0))
    return cosf, sinf


def _weight_inputs(W_qkv, W_o):
    """Per-core wqkvT [1024, 384] fp16 and woT [128, 1024] fp16 arrays."""
    wq, wo = [], []
    for c in range(NCORES):
        r0 = 128 * c
        wc = np.concatenate([W_qkv[r0:r0 + 128],
                             W_qkv[DM + r0:DM + r0 + 128],
                             W_qkv[2 * DM + r0:2 * DM + r0 + 128]], 0)
        wq.append(np.ascontiguousarray(wc.T).astype(np.float16))
        wo.append(np.ascontiguousarray(W_o[:, r0:r0 + 128].T).astype(np.float16))
    return wq, wo


class _Runtime:
    """Compiled NEFF + jitted dispatch + device-side input caches."""

    def __init__(self, s_len):
        import jax
        import jax.numpy as jnp
        from jax.sharding import Mesh, PartitionSpec, NamedSharding
        from jax.experimental.shard_map import shard_map
        from concourse.bass2jax import (_bass_exec_p, partition_id_tensor,
                                        install_neuronx_cc_hook)
        import concourse.mybir as mybir

        self.jax, self.jnp = jax, jnp
        self.s_len = s_len
        self.nc = _build(s_len)
        install_neuronx_cc_hook()
        nc = self.nc
        pname = nc.partition_id_tensor.name if nc.partition_id_tensor else None
        in_names, out_names, out_avals = [], [], []
        for alloc in nc.m.functions[0].allocations:
            if not isinstance(alloc, mybir.MemoryLocationSet):
                continue
            name = alloc.memorylocations[0].name
            if alloc.kind == "ExternalInput":
                if name != pname:
                    in_names.append(name)
            elif alloc.kind == "ExternalOutput":
                out_names.append(name)
                out_avals.append(jax.core.ShapedArray(
                    tuple(alloc.tensor_shape), mybir.dt.np(alloc.dtype)))
        self.in_names, self.out_names, self.out_avals = \
            in_names, out_names, out_avals
        n_params, n_outs = len(in_names), len(out_avals)
        all_in = list(in_names) + list(out_names) + ([pname] if pname else [])
        donate = tuple(range(n_params, n_params + n_outs))

        def _body(*args):
            operands = list(args)
            if pname:
                operands.append(partition_id_tensor())
            return tuple(_bass_exec_p.bind(
                *operands, out_avals=tuple(out_avals), in_names=tuple(all_in),
                out_names=tuple(out_names), lowering_input_output_aliases=(),
                sim_require_finite=True, sim_require_nnan=True, nc=nc))

        devices = jax.devices()[:NCORES]
        self.mesh = Mesh(np.asarray(devices), ("core",))
        P = PartitionSpec
        self.sh = NamedSharding(self.mesh, P("core"))
        self.exec_fn = jax.jit(
            shard_map(_body, mesh=self.mesh,
                      in_specs=(P("core"),) * (n_params + n_outs),
                      out_specs=(P("core"),) * n_outs, check_rep=False),
            donate_argnums=donate, keep_unused=True)
        self.mkz = jax.jit(
            lambda: tuple(jnp.zeros((NCORES * a.shape[0],) + a.shape[1:],
                                    a.dtype) for a in out_avals),
            out_shardings=tuple(self.sh for _ in out_avals))
        # replicate xT shards on device: [8*128, s] row-shards -> each core
        # holds the full [1024, s]; output layout = per-core blocks stacked
        self.agx = jax.jit(
            shard_map(lambda xs: jax.lax.all_gather(xs, "core", axis=0,
                                                    tiled=True),
                      mesh=self.mesh, in_specs=P("core"),
                      out_specs=P("core"), check_rep=False))
        # sum the 8 partial outputs in f32, return fp16 (one replica fetched)
        self.reduce = jax.jit(
            shard_map(lambda o: jax.lax.psum(o.astype(jnp.float32),
                                             "core").astype(jnp.float16),
                      mesh=self.mesh, in_specs=P("core"),
                      out_specs=P(), check_rep=False))
        # device caches for slow-changing inputs, keyed by host value
        self._wq_host = None
        self._wo_host = None
        self._tp_host = None
        self._dev = {}

    def stage_weights(self, W_qkv, W_o):
        if (self._wq_host is None
                or not np.array_equal(self._wq_host, W_qkv)
                or not np.array_equal(self._wo_host, W_o)):
            wq, wo = _weight_inputs(W_qkv, W_o)
            self._dev["wqkvT"] = self.jax.device_put(
                np.concatenate(wq, axis=0), self.sh)
            self._dev["woT"] = self.jax.device_put(
                np.concatenate(wo, axis=0), self.sh)
            self._wq_host = W_qkv.copy()
            self._wo_host = W_o.copy()

    def stage_tables(self, token_positions):
        tp = np.asarray(token_positions)
        if self._tp_host is None or not np.array_equal(self._tp_host, tp):
            cosf, sinf = _rope_tables(tp, self.s_len)
            self._dev["cosf"] = self.jax.device_put(
                np.concatenate([cosf] * NCORES, axis=0), self.sh)
            self._dev["sinf"] = self.jax.device_put(
                np.concatenate([sinf] * NCORES, axis=0), self.sh)
            self._tp_host = tp.copy()

    def stage_x(self, x):
        xT = np.ascontiguousarray(x.reshape(self.s_len, DM).T).astype(np.float16)
        x_sh = self.jax.device_put(xT, self.sh)      # 128 rows per core
        return self.agx(x_sh)                        # full xT on every core

    def run(self, x, token_positions, W_qkv, W_o):
        self.stage_weights(np.asarray(W_qkv), np.asarray(W_o))
        self.stage_tables(token_positions)
        xg = self.stage_x(np.asarray(x))
        feed = dict(self._dev)
        feed["xT"] = xg
        args = [feed[nm] for nm in self.in_names]
        outs = self.exec_fn(*args, *self.mkz())
        red = self.reduce(outs[self.out_names.index("outP")])
        return np.asarray(red).astype(np.float32).reshape(1, self.s_len, DM)


_RT = {}


def _get_rt(s_len):
    if s_len not in _RT:
        _RT[s_len] = _Runtime(s_len)
    return _RT[s_len]


def _kernel_fallback(x, token_positions, W_qkv, W_o, s_len):
    """Known-good slow path: replicated inputs, host-side partial sum."""
    from concourse.bass_utils import run_bass_kernel_spmd
    rt = _get_rt(s_len)
    cosf, sinf = _rope_tables(token_positions, s_len)
    xT = np.ascontiguousarray(x.reshape(s_len, DM).T).astype(np.float16)
    wq, wo = _weight_inputs(np.asarray(W_qkv), np.asarray(W_o))
    in_maps = [dict(xT=xT, wqkvT=wq[c], woT=wo[c], cosf=cosf, sinf=sinf)
               for c in range(NCORES)]
    res = run_bass_kernel_spmd(rt.nc, in_maps, core_ids=list(range(NCORES)))
    acc = np.zeros((s_len, DM), dtype=np.float32)
    for r in res.results:
        acc += np.asarray(r["outP"]).astype(np.float32)
    return acc.reshape(1, s_len, DM)


def kernel(x, token_positions, W_qkv, W_o):
    x = np.asarray(x)
    b, s_len, _ = x.shape
    assert b == 1
    # The axon-tunneled devices intermittently fault with
    # NRT_EXEC_UNIT_UNRECOVERABLE; retry, then fall back to the simple path.
    last_err = None
    for _attempt in range(3):
        try:
            rt = _get_rt(s_len)
            return rt.run(x, token_positions, W_qkv, W_o)
        except Exception as e:
            last_err = e
    try:
        return _kernel_fallback(x, token_positions, np.asarray(W_qkv),
                                np.asarray(W_o), s_len)
    except Exception:
        raise last_err
